# revision 60
# baseline (speedup 1.0000x reference)
"""Trainium2 Bass kernel for nn_MultiHeadAttention (B=2, S=2048, D=1024, H=16).

Sharding: 8 cores = 2 batches x 4 head-groups (4 heads each).
Each core receives host-shuffled activations x^T (layout [P, KC, S]) for its
batch plus its head-group's slices of the projection weights, computes
  Q^T,K^T = W^T x^T    (per-head [DK, S], heads stacked on partitions)
  V       = x W        (natural [S, DK] per head, + a ones column per head so
                        the AV matmul's row DK accumulates softmax denominators)
  scores^T[kv,q] = K Q^T / sqrt(DK), causal, exp (no max-sub: |s| < ~4)
  pav^T   = V_aug^T attn^T   (accumulated per 512-wide q-chunk in PSUM)
  rr      = exp(-ln Z)  ->  hcat^T = pav^T * partition_broadcast(rr)
  partial out = hcat^T^T @ (g * Wo rows)  -> [S, DOUT] fp16
The per-(batch,head) sigmoid gate g is folded into the Wo rows (per-partition
scale), so attention normalization never waits for the mean-pool reduction.
Host sums the 4 head-group partials per batch and adds bo.

vs the original 302 us baseline: one activation-table preload (kills 21
ACT_TABLE_LOAD thrashes), normalize via Ln/Exp + gpsimd partition_broadcast +
partition-shifted DVE writes (no broadcast matmuls, copies, or SBUF-SBUF
DMAs), projection bias-adds on DVE, mean-pools split scalar/DVE in FIFO-safe
pieces, fat contiguous DMA layouts, paired-kv 1024-wide exp reads, same-mode
matmul batching (64x128 scores vs 128x128 AV tiling modes drain the PE on
every switch), interleaved head-pair batches, per-chunk Q-projection
prefetch and out-projection, and fp16 partial output.
"""

import numpy as np

P = 128
CHUNK = 512  # q-chunk / matmul moving free dim

_BUILD_CACHE = {}


def _build(S, D, DOUT, HPC, DK, causal, bv_zero=False):
    """Emit the Bass program (same program for all cores; data differs)."""
    import concourse.bass as bass
    import concourse.mybir as mybir
    import concourse.tile as tile
    from concourse import bacc
    from concourse.bass import ds, ts
    from concourse.hw_specs import get_activation_tables

    fp32 = mybir.dt.float32
    fp16 = mybir.dt.float16
    bf16 = mybir.dt.bfloat16
    fp8 = mybir.dt.float8e4
    KC = D // P             # contraction k-chunks for projections
    GCOLS = HPC * DK        # this core's projection output width
    MT = GCOLS // P         # head-pair tiles (2 heads of DK=64 per tile)
    NCH = S // CHUNK        # q-chunks
    TPC = CHUNK // P        # kv tiles per q-chunk (4)
    NKV = S // P            # kv tiles total
    KC2 = GCOLS // P        # out-proj contraction chunks
    NOC = DOUT // CHUNK     # out-proj N chunks
    ST = S // P             # s-tiles
    J_ORDER = list(range(NCH - 1, -1, -1))  # largest chunk first
    HP2 = HPC // 2
    assert DK * 2 == P and GCOLS % P == 0

    Act = mybir.ActivationFunctionType
    nc = bacc.Bacc()

    xq_d = nc.declare_dram_parameter("xq", [P, KC, S], bf16, isOutput=False)
    xk_d = nc.declare_dram_parameter("xk", [P, KC, S], bf16, isOutput=False)
    xv_d = nc.declare_dram_parameter("xv", [P, KC, S], bf16, isOutput=False)
    wq_d = nc.declare_dram_parameter("wq", [P, KC, GCOLS], bf16, isOutput=False)
    wk_d = nc.declare_dram_parameter("wk", [P, KC, GCOLS], bf16, isOutput=False)
    wv_d = nc.declare_dram_parameter("wv", [P, KC, GCOLS], bf16, isOutput=False)
    wo_d = nc.declare_dram_parameter("wo", [P, KC2, DOUT], bf16, isOutput=False)
    bq_d = nc.declare_dram_parameter("bq", [P, MT], fp32, isOutput=False)
    bk_d = nc.declare_dram_parameter("bk", [P, MT], fp32, isOutput=False)
    bv_d = nc.declare_dram_parameter("bv", [1, GCOLS], bf16, isOutput=False)
    wgq_d = nc.declare_dram_parameter("wgq", [P, KC, HPC], fp32, isOutput=False)
    wgk_d = nc.declare_dram_parameter("wgk", [P, KC, HPC], fp32, isOutput=False)
    bg_d = nc.declare_dram_parameter("bg", [1, HPC], fp32, isOutput=False)
    mtri_d = nc.declare_dram_parameter("mtri", [P, P], bf16, isOutput=False)
    outp = nc.declare_dram_parameter("out", [S, DOUT], fp16, isOutput=True)

    scale = 1.0 / float(np.sqrt(DK))

    # natural_log_exp_and_others: covers Exp, Ln, Identity, Copy -- the only
    # scalar-engine functions this kernel uses.  Preloading it once stops the
    # compiler's per-function table churn (exp_and_others <-> natural_log).
    table_names = list(get_activation_tables(nc.m.arch))
    nle_set_id = table_names.index("natural_log_exp_and_others")

    with tile.TileContext(nc) as tc:
        nc.scalar.add_instruction(
            mybir.InstLoadActFuncSet(
                name=nc.get_next_instruction_name(),
                act_func_set_id=nle_set_id, ins=[], outs=[]))

        with (
            tc.tile_pool(name="persist", bufs=1) as pp,
            tc.tile_pool(name="work", bufs=1) as wkp,
            tc.tile_pool(name="ps", bufs=1, space="PSUM") as psp,
        ):
            # ------------- persistent tiles + weight/bias loads
            wq = pp.tile([P, KC, GCOLS], bf16, tag="wq")
            wk = pp.tile([P, KC, GCOLS], bf16, tag="wk")
            wv = pp.tile([P, KC, GCOLS], bf16, tag="wv")
            wo = pp.tile([P, KC2, DOUT], bf16, tag="wo")
            bq = pp.tile([P, MT], fp32, tag="bq")
            bk = pp.tile([P, MT], fp32, tag="bk")
            nc.sync.dma_start(wk[:], wk_d[:])
            nc.sync.dma_start(wq[:], wq_d[:])
            nc.sync.dma_start(bq[:], bq_d[:])
            nc.sync.dma_start(bk[:], bk_d[:])

            # c-major resident layout, fat contiguous per-partition DMA runs
            xq_r = pp.tile([P, KC, S], bf16, tag="xq_r")
            xk_r = pp.tile([P, KC, S], bf16, tag="xk_r")
            xv_r = pp.tile([P, KC, S], bf16, tag="xv_r")
            for c in range(0, KC, 2):
                nc.sync.dma_start(xk_r[:, c : c + 2, :], xk_d[:, c : c + 2, :])
            for c in range(0, KC, 2):
                nc.sync.dma_start(xq_r[:, c : c + 2, :], xq_d[:, c : c + 2, :])
            wgq = pp.tile([P, KC, HPC], fp32, tag="wgq")
            wgk = pp.tile([P, KC, HPC], fp32, tag="wgk")
            nc.sync.dma_start(wgq[:], wgq_d[:])
            nc.sync.dma_start(wgk[:], wgk_d[:])
            nc.sync.dma_start(wv[:], wv_d[:])
            for c in range(0, KC, 2):
                nc.sync.dma_start(xv_r[:, c : c + 2, :], xv_d[:, c : c + 2, :])
            nc.sync.dma_start(wo[:], wo_d[:])
            bv = pp.tile([1, GCOLS], bf16, tag="bv")
            nc.sync.dma_start(bv[:], bv_d[:])
            bg = pp.tile([1, HPC], fp32, tag="bg")
            nc.sync.dma_start(bg[:], bg_d[:])
            mtri = pp.tile([P, P], bf16, tag="mtri")
            nc.sync.dma_start(mtri[:], mtri_d[:])

            qt = pp.tile([P, MT, S], bf16, tag="qt")
            kt = pp.tile([P, MT, S], bf16, tag="kt")
            # vaug[:, st, h, :]: V at cols 0..DK-1, ones col at DK so the AV
            # matmul's output row DK accumulates the softmax denominators.
            vaug = pp.tile([P, ST, HPC, DK + 1], bf16, tag="vaug")
            hcat = pp.tile([P, KC2, S], bf16, tag="hcat")
            ones1 = pp.tile([1, P], bf16, tag="ones1")
            onesg = pp.tile([1, 1], fp32, tag="onesg")
            nc.any.memset(ones1[:], 1.0)
            nc.any.memset(onesg[:], 1.0)
            nc.any.memset(vaug[:, :, :, DK : DK + 1], 1.0)

            pooled_k = pp.tile([P, KC], fp32, tag="pooled_k")
            pool_nk = pp.tile([P, 2, KC, NCH], fp32, tag="pool_nk")
            kq_iter = iter([(c, n) for c in range(KC) for n in range(NCH)])

            # pooled column sums for the gate, in small pieces so they never
            # head-of-line-block evictions in the strict-FIFO engine queues:
            # xq rides the idle-early scalar engine (Copy+accum_out), xk goes
            # to DVE in [P, CHUNK] reduces.  wgq/wgk are pre-scaled 1/S.
            def pool_piece_q(c):
                scr = wkp.tile([P, S], bf16, tag="pscr", bufs=2, name="pscr")
                nc.scalar.activation(
                    scr[:], xq_r[:, c, :], Act.Copy,
                    accum_out=pool_nk[:, 0, c, 0:1])

            def pool_piece_k(m=1):
                for _ in range(m):
                    cn = next(kq_iter, None)
                    if cn is None:
                        return
                    c, n = cn
                    nc.vector.tensor_reduce(
                        pool_nk[:, 1, c, n : n + 1],
                        xk_r[:, c, ds(n * CHUNK, CHUNK)],
                        mybir.AxisListType.X, mybir.AluOpType.add)

            def proj_chunk(n, x_r, w_sb, b_sb, out_sb):
                nsl = ds(n * CHUNK, CHUNK)
                ps = psp.tile([P, 2, CHUNK], fp32, tag="sc", bufs=2, name="ps")
                for m in range(MT):
                    for k in range(KC):
                        nc.tensor.matmul(
                            ps[:, m, :], w_sb[:, k, ts(m, P)], x_r[:, k, nsl],
                            start=(k == 0), stop=(k == KC - 1))
                for m in range(MT):
                    nc.vector.tensor_scalar(
                        out_sb[:, m, nsl], ps[:, m, :], b_sb[:, m : m + 1],
                        None, mybir.AluOpType.add)

            # K projection (all chunks), then Q chunk 0.  xq pooling pieces
            # (scalar, whole rows) ride the pre-attention idle window; xk
            # pieces (DVE, chunk-sized) are emitted behind each K-chunk's
            # evictions so they can't starve the PSUM slots.
            for n in range(NCH):
                proj_chunk(n, xk_r, wk, bk, kt)
                pool_piece_q(2 * n)
                pool_piece_q(2 * n + 1)
                pool_piece_k(2)
            proj_chunk(J_ORDER[0], xq_r, wq, bq, qt)

            # V projection -> vaug (some xk pooling pieces woven in)
            for st in range(ST):
                pv = psp.tile([P, GCOLS], fp32, tag="pe" if st % 2 == 0 else "po",
                              bufs=2, name="pv")
                for k in range(KC):
                    nc.tensor.matmul(
                        pv[:], xv_r[:, k, ts(st, P)],
                        wv[:, k, :], start=(k == 0),
                        stop=(bv_zero and k == KC - 1))
                if not bv_zero:
                    nc.tensor.matmul(pv[:], ones1[0:1, 0:P], bv[:],
                                     start=False, stop=True)
                nc.vector.tensor_copy(
                    vaug[:, st, :, 0:DK],
                    pv.rearrange("p (h d) -> p h d", d=DK))
                if st % 2 == 1:
                    pool_piece_k(1)

            onesf = pp.tile([1, P], fp32, tag="onesf")
            nc.any.memset(onesf[:], 1.0)

            def emit_gate():
                # gate: sigmoid via exp/ln (stays in one table set):
                # g = exp(logits - ln(1 + exp(logits))) = sigmoid(logits).
                # Folded into the Wo rows (per-partition scale), so attention
                # normalization never waits for it.
                nc.vector.tensor_reduce(pooled_k[:], pool_nk[:, 1],
                                        mybir.AxisListType.X,
                                        mybir.AluOpType.add)
                psg = psp.tile([1, HPC], fp32, tag="pe", bufs=2, name="psg")
                for c in range(KC):
                    nc.tensor.matmul(psg[:], pool_nk[:, 0, c, 0:1],
                                     wgq[:, c, :],
                                     start=(c == 0), stop=False)
                for c in range(KC):
                    nc.tensor.matmul(psg[:], pooled_k[:, c : c + 1],
                                     wgk[:, c, :], start=False, stop=False)
                nc.tensor.matmul(psg[:], onesg[0:1, 0:1], bg[:],
                                 start=False, stop=True)
                ge = wkp.tile([1, HPC], fp32, tag="ge")
                gu = wkp.tile([1, HPC], fp32, tag="gu")
                gv = wkp.tile([1, HPC], fp32, tag="gv")
                lng0 = wkp.tile([1, HPC], fp32, tag="lng0")
                g0 = wkp.tile([1, HPC], fp32, tag="g0")
                nc.scalar.activation(ge[:], psg[:], Act.Exp)
                nc.vector.tensor_scalar(gu[:], ge[:], 1.0, None,
                                        mybir.AluOpType.add)
                nc.scalar.activation(gv[:], gu[:], Act.Ln)
                nc.vector.tensor_tensor(lng0[:], psg[:], gv[:],
                                        mybir.AluOpType.subtract)
                nc.scalar.activation(g0[:], lng0[:], Act.Exp)
                # broadcast g to all partitions, then scale wo rows in place:
                # row p of k2-tile belongs to head 2*k2 + (p >= DK)
                gbp = psp.tile([P, HPC], fp32, tag="po", bufs=2, name="gbp")
                nc.tensor.matmul(gbp[:], onesf[0:1, :], g0[:], start=True,
                                 stop=True)
                gb = wkp.tile([P, HPC], fp32, tag="gb")
                nc.vector.tensor_copy(gb[:], gbp[:])
                for k2 in range(KC2):
                    for lo, hi, h in ((0, DK, 2 * k2), (DK, P, 2 * k2 + 1)):
                        nc.vector.tensor_scalar(
                            wo[lo:hi, k2, :], wo[lo:hi, k2, :],
                            gb[lo:hi, h : h + 1], None, mybir.AluOpType.mult)

            # ------------- attention + out-projection, chunk by chunk
            # scores run in 64x128 PE-tiling mode (contraction 64, halves on
            # row-tiles 0/64); AV runs in 128x128 mode.  Batch several pairs
            # of each so the PE reconfigures (drains) once per batch instead
            # of on every matmul.  Chunks run largest-first so the kernel
            # tail is the shortest attention block.
            for jidx, j in enumerate(J_ORDER):
                jsl = ds(j * CHUNK, CHUNK)
                nkv_j = min(TPC * (j + 1), NKV) if causal else NKV
                npairs = (nkv_j + 1) // 2
                pavs = []
                for hp in range(MT):
                    pe_t = psp.tile([P, CHUNK], fp32, tag="pe", bufs=2,
                                    name="pe_t")
                    po_t = psp.tile([P, CHUNK], fp32, tag="po", bufs=2,
                                    name="po_t")
                    pavs.append((pe_t, po_t))
                # interleave the two head-pairs' batches so the PE always has
                # independent work while one batch's exps drain
                for bstart in range(0, npairs, 2):
                    for hp in range(MT):
                        pav = pavs[hp]
                        bpairs = min(2, npairs - bstart)
                        batch = []  # (half, at_t, subs)
                        # scores + exp phase (64-contraction mode)
                        for ipo in range(bpairs):
                            ip = bstart + ipo
                            sl_n = min(2, nkv_j - 2 * ip)
                            subs = []
                            for sl in range(sl_n):
                                i = 2 * ip + sl
                                t = i - TPC * j
                                if causal and t >= 0:
                                    subs.append((i, t, CHUNK - P * t, P * t))
                                else:
                                    subs.append((i, t, CHUNK, 0))
                            sc_p = [
                                psp.tile([P, 2, CHUNK], fp32, tag="sc",
                                         bufs=2, name="sc_t")
                                for _ in range(2)]
                            at_p = [
                                wkp.tile([P, 2, CHUNK], bf16, tag="at",
                                         bufs=8, name="at_t")
                                for _ in range(2)]
                            for sl, (i, t, Ni, qoff) in enumerate(subs):
                                for half in range(2):
                                    hsl = slice(half * DK, (half + 1) * DK)
                                    nc.tensor.matmul(
                                        sc_p[half][:, sl, :Ni],
                                        kt[hsl, hp, ts(i, P)],
                                        qt[hsl, hp, ds(j * CHUNK + qoff, Ni)],
                                        start=True, stop=True)
                            full = all(Ni == CHUNK for (i, t, Ni, qoff) in subs)
                            for half in range(2):
                                if full:
                                    nc.scalar.activation(
                                        at_p[half][:, 0:sl_n, :],
                                        sc_p[half][:, 0:sl_n, :],
                                        Act.Exp, scale=scale)
                                else:
                                    for sl, (i, t, Ni, qoff) in enumerate(subs):
                                        nc.scalar.activation(
                                            at_p[half][:, sl, :Ni],
                                            sc_p[half][:, sl, :Ni],
                                            Act.Exp, scale=scale)
                                for sl, (i, t, Ni, qoff) in enumerate(subs):
                                    if causal and t >= 0:
                                        nc.vector.tensor_mul(
                                            at_p[half][:, sl, 0:P],
                                            at_p[half][:, sl, 0:P], mtri[:])
                                batch.append((half, at_p[half], subs))
                        # AV phase (128-contraction mode)
                        for half, at_t, subs in batch:
                            for sl, (i, t, Ni, qoff) in enumerate(subs):
                                nc.tensor.matmul(
                                    pavs[hp][half][0 : DK + 1, ds(qoff, Ni)],
                                    vaug[:, i, 2 * hp + half, :],
                                    at_t[:, sl, :Ni],
                                    start=(i == 0), stop=(i == nkv_j - 1))
                        pool_piece_k(2)

                for hp in range(MT):
                    pav = pavs[hp]
                    # normalize + gate both halves of this head-pair.
                    # rr = exp(-ln Z + ln g) lands on partition 0 (DVE/ACT
                    # support shifted partition bases; partition_broadcast
                    # only works from base 0 on HW).
                    for half in range(2):
                        h = 2 * hp + half
                        lnz = wkp.tile([1, CHUNK], fp32, tag="lnz", bufs=4,
                                       name="lnz")
                        rr = wkp.tile([1, CHUNK], fp32, tag="rr", bufs=4,
                                      name="rr")
                        bc = wkp.tile([P, CHUNK], fp32, tag="bc", bufs=4,
                                      name="bc")
                        nc.scalar.activation(
                            lnz[0:1, :], pav[half][DK : DK + 1, :], Act.Ln)
                        nc.scalar.activation(
                            rr[0:1, :], lnz[0:1, :], Act.Exp, scale=-1.0)
                        nc.gpsimd.partition_broadcast(bc[0:DK, :], rr[0:1, :])
                        if half == 0:
                            nc.vector.tensor_mul(
                                hcat[0:DK, hp, jsl],
                                pav[half][0:DK, :], bc[0:DK, :])
                        else:
                            nc.vector.tensor_mul(
                                hcat[DK:P, hp, jsl],
                                pav[half][0:DK, :], bc[0:DK, :])

                # prefetch next q-chunk's Q projection
                if jidx + 1 < NCH:
                    proj_chunk(J_ORDER[jidx + 1], xq_r, wq, bq, qt)

                # out-projection (host adds bo).  wo carries the gate, which
                # needs all pooling pieces: emit the gate after attention j=1
                # and catch up on the deferred chunks then.
                def stage_c(jc):
                    for st in range(TPC * jc, TPC * (jc + 1)):
                        osb = wkp.tile([P, DOUT], fp16, tag="osb", bufs=4,
                                       name="osb")
                        for nh in range(NOC):
                            pc = psp.tile([P, CHUNK], fp32,
                                          tag="pe" if nh == 0 else "po",
                                          bufs=2, name="pc")
                            for k2 in range(KC2):
                                nc.tensor.matmul(
                                    pc[:], hcat[:, k2, ts(st, P)],
                                    wo[:, k2, ds(nh * CHUNK, CHUNK)],
                                    start=(k2 == 0), stop=(k2 == KC2 - 1))
                            nc.vector.tensor_copy(
                                osb[:, ds(nh * CHUNK, CHUNK)], pc[:])
                        nc.sync.dma_start(outp[ts(st, P), :], osb[:])

                if jidx == 1:
                    pool_piece_k(32)  # flush any leftovers
                    emit_gate()
                    stage_c(J_ORDER[0])
                    stage_c(J_ORDER[1])
                elif jidx >= 2:
                    stage_c(j)

    nc.compile()
    return nc


def _prep_core_inputs(query, key_, value, Wq, bq, Wk, bk, Wv, bv, Wg, bg, Wo,
                      b, g, S, D, HPC, DK):
    import ml_dtypes
    GCOLS = HPC * DK
    KC = D // P
    KC2 = GCOLS // P
    MT = GCOLS // P
    H0 = g * HPC
    cs = slice(H0 * DK, H0 * DK + GCOLS)
    f32 = np.float32
    bf16 = ml_dtypes.bfloat16
    c = np.ascontiguousarray

    def shuf_rows(a, nchunks):
        # [nchunks*P, N] -> [P, nchunks, N] with row r = chunk*P + p
        return c(a.reshape(nchunks, P, -1).transpose(1, 0, 2))

    return {
        "xq": shuf_rows(query[b].T.astype(bf16), KC),
        "xk": shuf_rows(key_[b].T.astype(bf16), KC),
        "xv": shuf_rows(value[b].T.astype(bf16), KC),
        "wq": shuf_rows(Wq[:, cs].astype(bf16), KC),
        "wk": shuf_rows(Wk[:, cs].astype(bf16), KC),
        "wv": shuf_rows(Wv[:, cs].astype(bf16), KC),
        "wo": shuf_rows(Wo[cs, :].astype(bf16), KC2),
        "bq": c(bq[cs].astype(f32).reshape(MT, P).T),
        "bk": c(bk[cs].astype(f32).reshape(MT, P).T),
        "bv": c(bv[cs].astype(bf16)[None, :]),
        "wgq": shuf_rows((Wg[:D, H0 : H0 + HPC] / S).astype(f32), KC),
        "wgk": shuf_rows((Wg[D:, H0 : H0 + HPC] / S).astype(f32), KC),
        "bg": c(bg[H0 : H0 + HPC].astype(f32)[None, :]),
        "mtri": np.triu(np.ones((P, P), bf16)),
    }


_last_results = None


def kernel(query, key_, value, mask, Wq, bq, Wk, bk, Wv, bv, Wo, bo, Wg, bg):
    global _last_results
    from concourse.bass_utils import run_bass_kernel_spmd

    query = np.asarray(query)
    key_ = np.asarray(key_)
    value = np.asarray(value)
    mask = np.asarray(mask)
    B, S, D = query.shape
    H = np.asarray(bg).shape[0]
    DK = D // H
    DOUT = np.asarray(Wo).shape[1]
    NC_ = 8
    GROUPS = NC_ // B
    HPC = H // GROUPS

    causal = bool(
        np.array_equal(mask[0, 0], np.tril(np.ones((S, S), bool)))
    )
    if not causal:
        assert mask.all(), "only causal or all-true masks supported"
    bv_zero = not np.asarray(bv).any()

    key = (S, D, DOUT, HPC, DK, causal, bv_zero)
    if key not in _BUILD_CACHE:
        _BUILD_CACHE[key] = _build(*key)
    nc = _BUILD_CACHE[key]

    in_maps = []
    for c in range(NC_):
        b, gidx = divmod(c, GROUPS)
        in_maps.append(_prep_core_inputs(
            query, key_, value, Wq, bq, Wk, bk, Wv, bv, Wg, bg, Wo,
            b, gidx, S, D, HPC, DK))

    res = run_bass_kernel_spmd(nc, in_maps, core_ids=list(range(NC_)))
    _last_results = res

    out = np.zeros((B, S, DOUT), np.float32)
    for c in range(NC_):
        b = c // GROUPS
        out[b] += res.results[c]["out"].astype(np.float32)
    out += np.asarray(bo).astype(np.float32)
    return out
